# revision 20
# baseline (speedup 1.0000x reference)
"""Distributed brute-force KNN (IndexFlatL2, K=3) + mean of gathered pred values.

Strategy (data-parallel over the memory bank N, queries replicated):
  - Device phase: per core, fp8e4m3 DoubleRow matmuls compute the corrected
    score s[b, n] = 2q.m_n - ||m_n||^2 directly in fp32 PSUM: the last 4 of
    the 1024 contraction dims are repurposed as a base-(256,32,4,0.5) digit
    encoding of -||m||^2 (digits are small integers, exact in fp8; residual
    <= 0.25; the 4 dropped data dims add ~N(0,4) noise, absorbed by the
    filter margin).  The DVE window-maxes each PSUM block (windows of 10
    rows) and the per-window maxima [B, 1250] stream back to DRAM.  No
    other device-side work: the Vector engine load (~136us) hides under the
    Tensor engine floor (~172us).
  - Host phase: rank the 8*1250 = 10000 candidate windows per query, take
    the top WSEL, exactly re-score their rows (fp64), take the true top-3,
    gather pred_values, return the mean.  Window capture margin (~50+ score
    units) dwarfs the fp8 scoring noise (~10).
"""

import sys
import types

import ml_dtypes
import numpy as np

try:  # bass_utils' axon trace path imports this unconditionally when
    import antenv.axon_hooks  # noqa: F401  # BASS_TRACE is set; stub it if absent
except ImportError:
    _stub = types.ModuleType("antenv.axon_hooks")
    _stub.get_axon_ntff_profile_hook = lambda: None
    _stub.set_axon_ntff_profile_hook = lambda hook: None
    sys.modules["antenv.axon_hooks"] = _stub

import concourse.bacc as bacc
import concourse.mybir as mybir
import concourse.tile as tile
from concourse import bass_utils

B = 1024            # queries
D = 1024            # embedding dim
N = 100000          # memory rows
NCORES = 8
NS = N // NCORES    # 12500 memory rows per core
BLK = 500           # matmul free-dim tile (fits one PSUM bank in fp32)
NBLK = NS // BLK    # 25 blocks per core
KT = D // 128       # 8 contraction subtiles
BCH = B // 128      # 8 query chunks of 128
WND = 10            # window width for the DVE windowed max
NWIN = NS // WND    # 1250 windows per core
WPB = BLK // WND    # 50 windows per block
K = 3
WSEL = 64           # windows exactly re-scored on host per query
# staged group widths: small first group so the PE starts early
GROUPS = [(0, 1), (1, 4), (5, 5), (10, 5), (15, 5), (20, 5)]
MAXW = 5
WARMUP_MM = 8       # dummy matmuls bridging the input-DMA wait: HAM clock warm

_CACHE = {}
LAST_RUN = None
LAST_TOP_IDX = None


def _build_program():
    nc = bacc.Bacc(
        "TRN2",
        target_bir_lowering=False,
        debug=False,
        enable_asserts=False,
        num_devices=NCORES,
    )
    f32 = mybir.dt.float32
    fp8 = mybir.dt.float8e4

    mT = nc.dram_tensor("mT", [D, NS], fp8, kind="ExternalInput").ap()
    # qT pre-permuted on host to [c*128+p, o*128+j]: per-chunk contiguous
    qT = nc.dram_tensor("qT", [B, KT * 128], fp8, kind="ExternalInput").ap()
    out_w = nc.dram_tensor("out_w", [B, NWIN], f32, kind="ExternalOutput").ap()

    mT_r = mT.rearrange("(o p) n -> p o n", p=128)
    qT_r = qT.rearrange("(c p) d -> p c d", p=128)
    ow_r = out_w.rearrange("(c p) j -> p c j", p=128)

    with tile.TileContext(nc) as tc:
        with (
            tc.tile_pool(name="const", bufs=1) as cpool,
            tc.tile_pool(name="mov", bufs=2) as movpool,
            tc.tile_pool(name="wm", bufs=4) as wmpool,
            tc.tile_pool(name="psum", bufs=8, space="PSUM") as pspool,
        ):
            # qt chunks on the SP ring, mov on the ACT ring; the first real
            # matmul only waits on chunk 0 + the first mov block
            qts = []
            for bc in range(BCH):
                qt_sb = cpool.tile([128, KT, 128], fp8, tag=f"qt{bc}")
                nc.sync.dma_start(
                    qt_sb, qT_r[:, bc, :].rearrange("p (o j) -> p o j", o=KT)
                )
                qts.append(qt_sb)

            # PE warm-up: dummy matmuls on memset tiles run during the input
            # DMA wait so the HAM clock gate is at 2.4 GHz for the real work
            wu_q = cpool.tile([128, 2, 128], fp8, tag="wuq")
            wu_m = cpool.tile([128, 2, BLK], fp8, tag="wum")
            nc.vector.memset(wu_q, 0.0)
            nc.vector.memset(wu_m, 0.0)
            wu_ps = pspool.tile([128, BLK], f32, tag="mm", name="mm_ps")
            for _ in range(WARMUP_MM):
                nc.tensor.matmul(
                    wu_ps,
                    lhsT=wu_q,
                    rhs=wu_m,
                    start=True,
                    stop=True,
                    perf_mode=mybir.MatmulPerfMode.DoubleRow,
                )

            for g0, w in GROUPS:
                n0 = g0 * BLK
                wn = w * BLK
                mov = movpool.tile([128, KT, MAXW * BLK], fp8, tag="mov")
                nc.scalar.dma_start(mov[:, :, :wn], mT_r[:, :, n0 : n0 + wn])
                for bc in range(BCH):
                    wt = wmpool.tile([128, MAXW * WPB], f32, tag="wt", name="wt")
                    for j in range(w):
                        ps = pspool.tile([128, BLK], f32, tag="mm", name="mm_ps")
                        for k in range(0, KT, 2):
                            nc.tensor.matmul(
                                ps,
                                lhsT=qts[bc][:, k : k + 2, :],
                                rhs=mov[:, k : k + 2, j * BLK : (j + 1) * BLK],
                                start=(k == 0),
                                stop=(k + 2 >= KT),
                                perf_mode=mybir.MatmulPerfMode.DoubleRow,
                            )
                        nc.vector.tensor_reduce(
                            wt[:, j * WPB : (j + 1) * WPB],
                            ps.rearrange("p (w t) -> p w t", t=WND),
                            axis=mybir.AxisListType.X,
                            op=mybir.AluOpType.max,
                            opt_input=False,
                        )
                    nc.sync.dma_start(
                        ow_r[:, bc, g0 * WPB : (g0 + w) * WPB], wt[:, : w * WPB]
                    )
    nc.compile()
    return nc


def kernel(h_query, memory_embeds, pred_values):
    global LAST_RUN, LAST_TOP_IDX
    q = np.ascontiguousarray(np.asarray(h_query, dtype=np.float32))
    m = np.ascontiguousarray(np.asarray(memory_embeds, dtype=np.float32))
    pv = np.asarray(pred_values, dtype=np.float32)

    # -||m||^2 folded into the contraction as 4 digit rows (exact to 0.125;
    # digits are small integers, scales are powers of two <= 240 = fp8e4m3 max)
    msq = np.einsum("nd,nd->n", m.astype(np.float64), m.astype(np.float64))
    a = np.rint(msq / 128.0)
    r = msq - 128.0 * a
    b = np.rint(r / 16.0)
    r -= 16.0 * b
    c = np.rint(r / 2.0)
    r -= 2.0 * c
    d = np.rint(r / 0.25)
    digit_rows = np.stack([-a, -b, -c, -d]).astype(np.float32)  # [4, N]

    fp8 = ml_dtypes.float8_e4m3
    qTs = np.empty((D, B), dtype=fp8)
    qTs[: D - 4] = (q.T[: D - 4] * np.float32(2.0)).astype(fp8)
    qTs[D - 4 :] = np.array([128.0, 16.0, 2.0, 0.25], dtype=np.float32)[
        :, None
    ].astype(fp8)
    # [o*128+p, c*128+j] -> [c*128+p, o*128+j]: per-chunk contiguous DMA lines
    qTp = np.ascontiguousarray(
        qTs.reshape(KT, 128, BCH, 128).transpose(2, 1, 0, 3).reshape(B, KT * 128)
    )
    mTs = np.empty((D, N), dtype=fp8)
    mTs[: D - 4] = m.T[: D - 4].astype(fp8)
    mTs[D - 4 :] = digit_rows.astype(fp8)

    if "nc" not in _CACHE:
        _CACHE["nc"] = _build_program()
    nc = _CACHE["nc"]

    in_maps = []
    for cix in range(NCORES):
        sl = slice(cix * NS, (cix + 1) * NS)
        in_maps.append({"mT": np.ascontiguousarray(mTs[:, sl]), "qT": qTp})

    res = bass_utils.run_bass_kernel_spmd(nc, in_maps, core_ids=list(range(NCORES)))
    LAST_RUN = res
    results = res.results

    # [B, 8*1250] corrected window scores; window w covers rows
    # [(w // NWIN) * NS + (w % NWIN) * WND, +WND)
    wall = np.concatenate([r["out_w"] for r in results], axis=1)

    sel = np.argpartition(-wall, WSEL, axis=1)[:, :WSEL]      # [B, WSEL]
    core = sel // NWIN
    rows = (core * NS + (sel % NWIN) * WND)[:, :, None] + np.arange(WND)[
        None, None, :
    ]
    cidx = rows.reshape(B, WSEL * WND)                         # candidate rows

    # exact fp64 re-score of candidate rows, chunked over queries
    q64 = q.astype(np.float64)
    m64 = m.astype(np.float64)
    msq64 = msq
    top_idx = np.empty((B, K), dtype=np.int64)
    CH = 128
    for b0 in range(0, B, CH):
        ci = cidx[b0 : b0 + CH]                                # [CH, WSEL*WND]
        mg = m64[ci]                                           # [CH, R, D]
        s = 2.0 * np.einsum("bd,bkd->bk", q64[b0 : b0 + CH], mg)
        s -= msq64[ci]
        pick = np.argpartition(-s, K, axis=1)[:, :K]
        top_idx[b0 : b0 + CH] = np.take_along_axis(ci, pick, axis=1)
    LAST_TOP_IDX = top_idx
    y = pv[top_idx].astype(np.float64).mean()
    return np.float32(y)


# revision 25
# speedup vs baseline: 1.0148x; 1.0148x over previous
"""Distributed brute-force KNN (IndexFlatL2, K=3) + mean of gathered pred values.

Strategy (data-parallel over the memory bank N, queries replicated):
  - Device phase: per core, fp8e4m3 DoubleRow matmuls compute the corrected
    score s[b, n] = 2q.m_n - ||m_n||^2 directly in fp32 PSUM: the last 4 of
    the 1024 contraction dims are repurposed as a base-(256,32,4,0.5) digit
    encoding of -||m||^2 (digits are small integers, exact in fp8; residual
    <= 0.25; the 4 dropped data dims add ~N(0,4) noise, absorbed by the
    filter margin).  The DVE window-maxes each PSUM block (windows of 10
    rows) and the per-window maxima [B, 1250] stream back to DRAM.  No
    other device-side work: the Vector engine load (~136us) hides under the
    Tensor engine floor (~172us).
  - Host phase: rank the 8*1250 = 10000 candidate windows per query, take
    the top WSEL, exactly re-score their rows (fp64), take the true top-3,
    gather pred_values, return the mean.  Window capture margin (~50+ score
    units) dwarfs the fp8 scoring noise (~10).
"""

import sys
import types

import ml_dtypes
import numpy as np

try:  # bass_utils' axon trace path imports this unconditionally when
    import antenv.axon_hooks  # noqa: F401  # BASS_TRACE is set; stub it if absent
except ImportError:
    _stub = types.ModuleType("antenv.axon_hooks")
    _stub.get_axon_ntff_profile_hook = lambda: None
    _stub.set_axon_ntff_profile_hook = lambda hook: None
    sys.modules["antenv.axon_hooks"] = _stub

import concourse.bacc as bacc
import concourse.mybir as mybir
import concourse.tile as tile
from concourse import bass_utils

B = 1024            # queries
D = 1024            # embedding dim
N = 100000          # memory rows
NCORES = 8
NS = N // NCORES    # 12500 memory rows per core
BLK = 500           # matmul free-dim tile (fits one PSUM bank in fp32)
NBLK = NS // BLK    # 25 blocks per core
KT = D // 128       # 8 contraction subtiles
BCH = B // 128      # 8 query chunks of 128
WND = 10            # window width for the DVE windowed max
NWIN = NS // WND    # 1250 windows per core
WPB = BLK // WND    # 50 windows per block
K = 3
WSEL = 64           # windows exactly re-scored on host per query
# staged group widths: small first group so the PE starts early
GROUPS = [(0, 1), (1, 4), (5, 5), (10, 5), (15, 5), (20, 5)]
MAXW = 5
WARMUP_MM = 11      # dummy matmuls bridging the input-DMA wait: HAM clock warm

_CACHE = {}
LAST_RUN = None
LAST_TOP_IDX = None


def _build_program():
    nc = bacc.Bacc(
        "TRN2",
        target_bir_lowering=False,
        debug=False,
        enable_asserts=False,
        num_devices=NCORES,
    )
    f32 = mybir.dt.float32
    fp8 = mybir.dt.float8e4

    mT = nc.dram_tensor("mT", [D, NS], fp8, kind="ExternalInput").ap()
    # qT pre-permuted on host to [p, o*B + b] so the DMA has 8KB lines
    qT = nc.dram_tensor("qT", [128, KT * B], fp8, kind="ExternalInput").ap()
    out_w = nc.dram_tensor("out_w", [B, NWIN], f32, kind="ExternalOutput").ap()

    mT_r = mT.rearrange("(o p) n -> p o n", p=128)
    ow_r = out_w.rearrange("(c p) j -> p c j", p=128)

    with tile.TileContext(nc) as tc:
        with (
            tc.tile_pool(name="const", bufs=1) as cpool,
            tc.tile_pool(name="mov", bufs=2) as movpool,
            tc.tile_pool(name="wm", bufs=4) as wmpool,
            tc.tile_pool(name="psum", bufs=8, space="PSUM") as pspool,
        ):
            # qt on the SP ring, mov on the ACT ring: the two startup
            # transfers land in parallel
            qt_sb = cpool.tile([128, KT, B], fp8, tag="qt")
            nc.sync.dma_start(qt_sb, qT.rearrange("p (o b) -> p o b", o=KT))

            # PE warm-up: dummy matmuls on memset tiles run during the input
            # DMA wait so the HAM clock gate is at 2.4 GHz for the real work
            wu_q = cpool.tile([128, 2, 128], fp8, tag="wuq")
            wu_m = cpool.tile([128, 2, BLK], fp8, tag="wum")
            nc.vector.memset(wu_q, 0.0)
            nc.vector.memset(wu_m, 0.0)
            wu_ps = pspool.tile([128, BLK], f32, tag="mm", name="mm_ps")
            for _ in range(WARMUP_MM):
                nc.tensor.matmul(
                    wu_ps,
                    lhsT=wu_q,
                    rhs=wu_m,
                    start=True,
                    stop=True,
                    perf_mode=mybir.MatmulPerfMode.DoubleRow,
                )

            for g0, w in GROUPS:
                n0 = g0 * BLK
                wn = w * BLK
                mov = movpool.tile([128, KT, MAXW * BLK], fp8, tag="mov")
                nc.scalar.dma_start(mov[:, :, :wn], mT_r[:, :, n0 : n0 + wn])
                for bc in range(BCH):
                    wt = wmpool.tile([128, MAXW * WPB], f32, tag="wt", name="wt")
                    for j in range(w):
                        ps = pspool.tile([128, BLK], f32, tag="mm", name="mm_ps")
                        for k in range(0, KT, 2):
                            nc.tensor.matmul(
                                ps,
                                lhsT=qt_sb[:, k : k + 2, bc * 128 : (bc + 1) * 128],
                                rhs=mov[:, k : k + 2, j * BLK : (j + 1) * BLK],
                                start=(k == 0),
                                stop=(k + 2 >= KT),
                                perf_mode=mybir.MatmulPerfMode.DoubleRow,
                            )
                        nc.vector.tensor_reduce(
                            wt[:, j * WPB : (j + 1) * WPB],
                            ps.rearrange("p (w t) -> p w t", t=WND),
                            axis=mybir.AxisListType.X,
                            op=mybir.AluOpType.max,
                            opt_input=False,
                        )
                    nc.sync.dma_start(
                        ow_r[:, bc, g0 * WPB : (g0 + w) * WPB], wt[:, : w * WPB]
                    )
    nc.compile()
    return nc


def kernel(h_query, memory_embeds, pred_values):
    global LAST_RUN, LAST_TOP_IDX
    q = np.ascontiguousarray(np.asarray(h_query, dtype=np.float32))
    m = np.ascontiguousarray(np.asarray(memory_embeds, dtype=np.float32))
    pv = np.asarray(pred_values, dtype=np.float32)

    # -||m||^2 folded into the contraction as 4 digit rows (exact to 0.125;
    # digits are small integers, scales are powers of two <= 240 = fp8e4m3 max)
    msq = np.einsum("nd,nd->n", m.astype(np.float64), m.astype(np.float64))
    a = np.rint(msq / 128.0)
    r = msq - 128.0 * a
    b = np.rint(r / 16.0)
    r -= 16.0 * b
    c = np.rint(r / 2.0)
    r -= 2.0 * c
    d = np.rint(r / 0.25)
    digit_rows = np.stack([-a, -b, -c, -d]).astype(np.float32)  # [4, N]

    fp8 = ml_dtypes.float8_e4m3
    qTs = np.empty((D, B), dtype=fp8)
    qTs[: D - 4] = (q.T[: D - 4] * np.float32(2.0)).astype(fp8)
    qTs[D - 4 :] = np.array([128.0, 16.0, 2.0, 0.25], dtype=np.float32)[
        :, None
    ].astype(fp8)
    # [o*128+p, b] -> [p, o*B+b]: contiguous 8KB DMA lines per partition
    qTp = np.ascontiguousarray(
        qTs.reshape(KT, 128, B).transpose(1, 0, 2).reshape(128, KT * B)
    )
    mTs = np.empty((D, N), dtype=fp8)
    mTs[: D - 4] = m.T[: D - 4].astype(fp8)
    mTs[D - 4 :] = digit_rows.astype(fp8)

    if "nc" not in _CACHE:
        _CACHE["nc"] = _build_program()
    nc = _CACHE["nc"]

    in_maps = []
    for cix in range(NCORES):
        sl = slice(cix * NS, (cix + 1) * NS)
        in_maps.append({"mT": np.ascontiguousarray(mTs[:, sl]), "qT": qTp})

    res = bass_utils.run_bass_kernel_spmd(nc, in_maps, core_ids=list(range(NCORES)))
    LAST_RUN = res
    results = res.results

    # [B, 8*1250] corrected window scores; window w covers rows
    # [(w // NWIN) * NS + (w % NWIN) * WND, +WND)
    wall = np.concatenate([r["out_w"] for r in results], axis=1)

    sel = np.argpartition(-wall, WSEL, axis=1)[:, :WSEL]      # [B, WSEL]
    core = sel // NWIN
    rows = (core * NS + (sel % NWIN) * WND)[:, :, None] + np.arange(WND)[
        None, None, :
    ]
    cidx = rows.reshape(B, WSEL * WND)                         # candidate rows

    # exact fp64 re-score of candidate rows, chunked over queries
    q64 = q.astype(np.float64)
    m64 = m.astype(np.float64)
    msq64 = msq
    top_idx = np.empty((B, K), dtype=np.int64)
    CH = 128
    for b0 in range(0, B, CH):
        ci = cidx[b0 : b0 + CH]                                # [CH, WSEL*WND]
        mg = m64[ci]                                           # [CH, R, D]
        s = 2.0 * np.einsum("bd,bkd->bk", q64[b0 : b0 + CH], mg)
        s -= msq64[ci]
        pick = np.argpartition(-s, K, axis=1)[:, :K]
        top_idx[b0 : b0 + CH] = np.take_along_axis(ci, pick, axis=1)
    LAST_TOP_IDX = top_idx
    y = pv[top_idx].astype(np.float64).mean()
    return np.float32(y)
